# revision 9
# baseline (speedup 1.0000x reference)
"""Causal self-attention (single head) on 8 TRN2 NeuronCores.

Sharding: data-parallel over batch (4) x query-interleave (2).
Core c handles batch b = c//2 and the 8 query blocks J_BLOCKS[c%2]
(chosen so slot s covers global blocks {2s, 2s+1} and both cores of a
batch have equal causal work).  Keys are host-permuted rank-order:
slot s columns [256s, 256s+256) = [J0[s] block | J1[s] block], so for
query block t only the diagonal slot t needs a causal mask; all keys
before it are strictly past.

Host-fused weights (f32 GEMMs):
  scores = x (Wq^T Wk) x^T            -> Wqk   (K never materializes)
  out    = softmax(.) x (Wo Wv)^T + (Wo bv + bo)   -> Wvo, bvo

Phase A: d-major projection rounds (4 PSUM banks per round, two
alternating bank groups) so input DMA paces compute chunk-by-chunk.
Phase B: per-tile softmax straight out of PSUM (no max subtraction --
scaled scores are bounded by ~6), diagonal-only mask add, 3-stage
software pipeline S(t) / T+A(t-1) over query blocks, big blocks first.

Compute dtype: bf16 matmuls (f32 PSUM accumulate), f32 softmax stats.
"""

from contextlib import ExitStack

import numpy as np
import ml_dtypes

B, S, D = 4, 2048, 1024
P = 128
ND = D // P  # 8 d (contraction) chunks
NE = D // P  # 8 output-feature chunks
NSK = S // P  # 16 key chunks
NQB = 8  # query blocks per core
SQH = NQB * P  # 1024 queries per core
J_BLOCKS = (
    [0, 3, 4, 7, 8, 11, 12, 15],
    [1, 2, 5, 6, 9, 10, 13, 14],
)
COVS = [256 * (t + 1) for t in range(NQB)]  # key coverage per block
SCALE = 1.0 / np.sqrt(np.float32(D))  # 1/32
NEG_BIG = -1.0e30
CPAK = NE + D + S  # packed f32 consts: bq_t | bo_b | mdiag
N_WARM = 60  # PE warm-up matmuls (HAM un-throttle before real work)

_NC = None


def _score_spans(t):
    """(offset, width, diag_lo) score psum tiles covering [0, 256(t+1)).

    diag_lo = column (within the tile) where the diagonal slot starts;
    width for cols >= diag_lo needs the mask add, below is mask-free.
    """
    cov = COVS[t]
    dstart = cov - 256
    spans = []
    off = 0
    while off < cov:
        wdt = min(512, cov - off)
        dlo = min(max(dstart - off, 0), wdt)
        spans.append((off, wdt, dlo))
        off += wdt
    return spans


def _emit(nc, tc, dr, out_d):
    import concourse.bass as bass  # noqa: F401
    from concourse import mybir

    BF = mybir.dt.bfloat16
    F32 = mybir.dt.float32
    AF = mybir.ActivationFunctionType
    Alu = mybir.AluOpType

    with ExitStack() as ctx:
        const = ctx.enter_context(tc.tile_pool(name="const", bufs=1))
        # packed f32 consts: bq head first (needed at first Q' evict),
        # bo+mdiag later (phase B), both off the hot DMA rings
        cpak = const.tile([P, CPAK], F32)
        bq = cpak[:, 0:NE]
        bo = cpak[:, NE : NE + D]
        mdiag = cpak[:, NE + D : NE + D + S]
        ident = const.tile([P, P], BF)
        warm = const.tile([P, P], BF)
        nc.gpsimd.memset(warm[:], 0.0)
        # persistent activation storage
        qt_pool = ctx.enter_context(tc.tile_pool(name="qt", bufs=NE))
        v_pool = ctx.enter_context(tc.tile_pool(name="v", bufs=1))
        xt_pool = ctx.enter_context(tc.tile_pool(name="xt", bufs=ND))
        xm_pool = ctx.enter_context(tc.tile_pool(name="xm", bufs=ND))
        QT = [qt_pool.tile([P, SQH], BF, name="qt") for _ in range(NE)]
        VT = v_pool.tile([P, NSK * D], BF)  # V[k-chunk] at cols [1024k,1024k+1024)
        XT = [xt_pool.tile([P, S], BF, name="xt") for _ in range(ND)]
        XM = [xm_pool.tile([P, SQH], BF, name="xm") for _ in range(ND)]

        # ---------------- phase A: projections ----------------
        with ExitStack() as actx:
            w_pool = actx.enter_context(tc.tile_pool(name="w", bufs=2))
            psp = actx.enter_context(tc.tile_pool(name="psp", bufs=8, space="PSUM"))
            WQ = w_pool.tile([P, ND * D], BF)  # Wqk[d-chunk] at cols [1024d, ...)
            WV = w_pool.tile([P, ND * D], BF)  # Wvo^T[d-chunk] at cols [1024d, ...)

            # first wave: x_mine + wqd interleaved by d across the 3 DMA
            # rings (sync/gpsimd/scalar), ordered by first use, so the
            # d-major Q' rounds never starve; second wave: xT / wv / consts
            rings = [nc.sync, nc.gpsimd, nc.scalar]

            def ring(i):
                return rings[i % 3]

            first = []  # (ring_idx, dst, src) round-robin by d order
            for d in range(ND):
                first.append((XM[d][:], dr["xm"][d]))
                first.append((WQ[:, d * D : (d + 1) * D], dr["wq"][d]))
            for i, (dst, src) in enumerate(first):
                ring(i).dma_start(dst, src)
            nc.scalar.dma_start(cpak[:, :NE], dr["cpak"][:, :NE])
            for d in range(ND):
                ring(d).dma_start(XT[d][:], dr["xT"][d])
            nc.scalar.dma_start(WV[:, : 4 * D], dr["wv"][:, : 4 * D])
            nc.sync.dma_start(WV[:, 4 * D :], dr["wv"][:, 4 * D :])
            nc.gpsimd.dma_start(ident[:], dr["ident"])
            nc.gpsimd.dma_start(cpak[:, NE:], dr["cpak"][:, NE:])

            # PE warm-up: dummy matmuls on a zero tile keep the HAM busy
            # window alive until real operands land (cold PE = 1.2 GHz).
            wps = psp.tile([P, 512], F32, tag="ps", name="ps")
            for _ in range(N_WARM):
                nc.tensor.matmul(wps[:, :P], warm[:], warm[:], start=True, stop=True)

            # QT[e] = (x_mine Wqk)^T[e-chunk] + bq -> [128 e, 1024 q] bf16
            # d-major rounds of 4 psum banks (two groups alternate)
            for qh, eh in ((0, 0), (1, 0), (0, 1), (1, 1)):
                pss = [psp.tile([P, 512], F32, tag="ps", name="ps") for _ in range(4)]
                for d in range(ND):
                    for ei in range(4):
                        e = eh * 4 + ei
                        nc.tensor.matmul(
                            pss[ei][:],
                            WQ[:, d * D + e * P : d * D + (e + 1) * P],
                            XM[d][:, qh * 512 : (qh + 1) * 512],
                            start=(d == 0),
                            stop=(d == ND - 1),
                        )
                for ei in range(4):
                    e = eh * 4 + ei
                    nc.scalar.activation(
                        QT[e][:, qh * 512 : (qh + 1) * 512],
                        pss[ei][:],
                        AF.Identity,
                        bias=bq[:, e : e + 1],
                        scale=1.0,
                    )

            # V[s] = (x Wvo^T)[s-chunk] -> [128 s, 1024 e] bf16 (bvo in bo)
            for sp2 in range(NSK // 2):
                pss = [psp.tile([P, 512], F32, tag="ps", name="ps") for _ in range(4)]
                for d in range(ND):
                    for si in range(2):
                        s = sp2 * 2 + si
                        for nt in range(2):
                            nc.tensor.matmul(
                                pss[si * 2 + nt][:],
                                XT[d][:, s * P : (s + 1) * P],
                                WV[:, d * D + nt * 512 : d * D + (nt + 1) * 512],
                                start=(d == 0),
                                stop=(d == ND - 1),
                            )
                for si in range(2):
                    s = sp2 * 2 + si
                    for nt in range(2):
                        nc.scalar.activation(
                            VT[:, s * D + nt * 512 : s * D + (nt + 1) * 512],
                            pss[si * 2 + nt][:],
                            AF.Copy,
                        )

        # ---------------- phase B: attention + output projection ----------------
        # S(t): score matmuls + per-tile exp from PSUM (diag tile: mask add)
        # T(t): PE transposes of weights;  A(t): attend + evict + bias + store
        # PE order: S7 | S6 T7 A7 | S5 T6 A6 | ... | S0 T1 A1 | T0 A0
        with ExitStack() as bctx:
            wp = bctx.enter_context(tc.tile_pool(name="w_sb", bufs=2))
            wtp = bctx.enter_context(tc.tile_pool(name="wt_sb", bufs=3))
            outp = bctx.enter_context(tc.tile_pool(name="out_sb", bufs=2))
            accp = bctx.enter_context(tc.tile_pool(name="accp", bufs=8))
            sdp = bctx.enter_context(tc.tile_pool(name="sdp", bufs=2))
            rip = bctx.enter_context(tc.tile_pool(name="rip", bufs=3))
            ps_s = bctx.enter_context(tc.tile_pool(name="ps_s", bufs=3, space="PSUM"))
            ps_t = bctx.enter_context(tc.tile_pool(name="ps_t", bufs=2, space="PSUM"))
            ps_a = bctx.enter_context(tc.tile_pool(name="ps_a", bufs=3, space="PSUM"))

            def emit_scores(t):
                cov = COVS[t]
                w_sb = wp.tile([P, cov], BF, tag="w")
                accs = []
                for off, wdt, dlo in _score_spans(t):
                    ps = ps_s.tile([P, wdt], F32, tag="ps_s")
                    for e in range(NE):
                        nc.tensor.matmul(
                            ps[:],
                            QT[e][:, t * P : (t + 1) * P],
                            XT[e][:, off : off + wdt],
                            start=(e == 0),
                            stop=(e == NE - 1),
                        )
                    if dlo > 0:  # mask-free part: exp straight from psum
                        acc = accp.tile([P, 1], F32, tag="acc", name="acc")
                        nc.scalar.activation(
                            w_sb[:, off : off + dlo],
                            ps[:, 0:dlo],
                            AF.Exp,
                            bias=0.0,
                            scale=float(SCALE),
                            accum_out=acc[:],
                        )
                        accs.append(acc)
                    if dlo < wdt:  # diagonal slot: add causal mask, then exp
                        sd = sdp.tile([P, 256], F32, tag="sd", name="sd")
                        nc.vector.tensor_tensor(
                            sd[:, : wdt - dlo],
                            ps[:, dlo:wdt],
                            mdiag[:, off + dlo - (cov - 256) + t * 256 : off
                                  + wdt - (cov - 256) + t * 256],
                            op=Alu.add,
                        )
                        acc = accp.tile([P, 1], F32, tag="acc", name="acc")
                        nc.scalar.activation(
                            w_sb[:, off + dlo : off + wdt],
                            sd[:, : wdt - dlo],
                            AF.Exp,
                            bias=0.0,
                            scale=float(SCALE),
                            accum_out=acc[:],
                        )
                        accs.append(acc)
                lsum = accs[0]
                for acc in accs[1:]:
                    nc.vector.tensor_tensor(lsum[:], lsum[:], acc[:], op=Alu.add)
                rinv = rip.tile([P, 1], F32, tag="rinv")
                nc.vector.reciprocal(rinv[:], lsum[:])
                return {"t": t, "w_sb": w_sb, "rinv": rinv}

            def emit_attend(st):
                t = st["t"]
                cov = COVS[t]
                K = cov // P
                w_sb, rinv = st["w_sb"], st["rinv"]
                # weight transposes on PE (matmul transpose mode)
                wT = wtp.tile([P, cov], BF, tag="wt")
                for k in range(K):
                    pt = ps_t.tile([P, P], BF, tag="pt")
                    nc.tensor.transpose(pt[:], w_sb[:, k * P : (k + 1) * P], ident[:])
                    nc.vector.tensor_copy(wT[:, k * P : (k + 1) * P], pt[:])
                outsb = outp.tile([P, D], F32, tag="o")
                last = t == 0
                for nt in range(2):
                    pa = ps_a.tile([P, 512], F32, tag="pa")
                    for k in range(K):
                        nc.tensor.matmul(
                            pa[:],
                            wT[:, k * P : (k + 1) * P],
                            VT[:, k * D + nt * 512 : k * D + (nt + 1) * 512],
                            start=(k == 0),
                            stop=(k == K - 1),
                        )
                    # out = psum * rinv (softmax normalize) then + bvo
                    nc.scalar.activation(
                        outsb[:, nt * 512 : (nt + 1) * 512],
                        pa[:],
                        AF.Copy,
                        bias=0.0,
                        scale=rinv[:],
                    )
                    nc.vector.tensor_tensor(
                        outsb[:, nt * 512 : (nt + 1) * 512],
                        outsb[:, nt * 512 : (nt + 1) * 512],
                        bo[:, nt * 512 : (nt + 1) * 512],
                        op=Alu.add,
                    )
                    # final block: split store 4-way for a short tail
                    if last:
                        for h in range(2):
                            lo = nt * 512 + h * 256
                            (nc.sync if h == 0 else nc.gpsimd).dma_start(
                                out_d[t][:, lo : lo + 256], outsb[:, lo : lo + 256]
                            )
                    else:
                        (nc.sync if nt == 0 else nc.gpsimd).dma_start(
                            out_d[t][:, nt * 512 : (nt + 1) * 512],
                            outsb[:, nt * 512 : (nt + 1) * 512],
                        )

            order = list(range(NQB - 1, -1, -1))  # big blocks first
            states = []
            for i, t in enumerate(order):
                states.append(emit_scores(t))
                if i >= 1:
                    emit_attend(states[i - 1])
            emit_attend(states[-1])


def build_nc():
    """Build + compile the SPMD Bass program (cached)."""
    global _NC
    if _NC is not None:
        return _NC
    from concourse import bacc, mybir
    import concourse.tile as tile

    BF = mybir.dt.bfloat16
    F32 = mybir.dt.float32

    nc = bacc.Bacc(
        "TRN2", target_bir_lowering=False, debug=False, enable_asserts=False
    )
    dr = {}

    def din(name, shape, dt):
        dr[name] = nc.dram_tensor(name, shape, dt, kind="ExternalInput").ap()

    din("xm", (ND, P, SQH), BF)
    din("xT", (ND, P, S), BF)
    din("wq", (ND, P, D), BF)
    din("wv", (P, ND * D), BF)
    din("ident", (P, P), BF)
    din("cpak", (P, CPAK), F32)
    out_d = nc.dram_tensor("out_c", (NQB, P, D), F32, kind="ExternalOutput").ap()

    with tile.TileContext(nc) as tc:
        _emit(nc, tc, dr, out_d)
    nc.compile()
    _NC = nc
    return nc


def make_in_maps(x, Wq, bq, Wk, bk, Wv, bv, Wo, bo):
    """Host-side sharding: per-core input dicts (bf16 compute operands)."""
    bf16 = ml_dtypes.bfloat16
    f32 = np.float32

    # host-fused weights (f32 GEMMs, exact up to fp32):
    #   scores = (x Wq^T)(x Wk^T)^T = x (Wq^T Wk) x^T       -> Wqk
    #   out    = softmax(..) (x Wv^T) Wo^T = softmax(..) x (Wo Wv)^T
    Wqk = Wq.T.astype(f32) @ Wk.astype(f32)  # [d, e]; Q' = x @ Wqk
    Wvo = Wo.astype(f32) @ Wv.astype(f32)  # [e_out, d]; V' = x @ Wvo^T
    bvo = Wo.astype(f32) @ bv.astype(f32) + bo.astype(f32)
    wq_c = np.ascontiguousarray(Wqk.reshape(ND, P, D)).astype(bf16)
    wv_c = np.ascontiguousarray(
        Wvo.T.reshape(ND, P, D).transpose(1, 0, 2).reshape(P, ND * D)
    ).astype(bf16)
    bq_t = np.ascontiguousarray(bq.reshape(NE, P).T).astype(f32)
    bo_b = np.broadcast_to(bvo, (P, D))
    ident = np.eye(P, dtype=bf16)

    J0, J1 = J_BLOCKS
    # rank-order key permutation: slot s = [J0[s] block | J1[s] block]
    perm = np.concatenate(
        [
            np.r_[P * J0[s] : P * (J0[s] + 1), P * J1[s] : P * (J1[s] + 1)]
            for s in range(NQB)
        ]
    )
    kpos = perm.astype(np.int64)  # [2048] absolute key positions

    in_maps = []
    for c in range(8):
        b, j = c // 2, c % 2
        blocks = J_BLOCKS[j]
        xTb = np.ascontiguousarray(x[b].T[:, perm])  # [D, S] permuted keys
        mycols = np.concatenate(
            [np.r_[P * g : P * (g + 1)] for g in blocks]
        )
        xmb = np.ascontiguousarray(x[b].T[:, mycols])  # [D, 1024] my tokens
        # diagonal-slot causal mask: block t vs keys [256t, 256t+256)
        mdiag = np.zeros((P, S), f32)
        for t in range(NQB):
            qpos = P * blocks[t] + np.arange(P)  # [128] my query positions
            kd = kpos[256 * t : 256 * (t + 1)]  # [256] diag key positions
            mdiag[:, 256 * t : 256 * (t + 1)] = np.where(
                kd[None, :] > qpos[:, None], NEG_BIG, 0.0
            )
        cpak = np.concatenate([bq_t, bo_b, mdiag], axis=1)
        assert cpak.shape == (P, CPAK)
        in_maps.append(
            {
                "xm": xmb.reshape(ND, P, SQH).astype(bf16),
                "xT": xTb.reshape(ND, P, S).astype(bf16),
                "wq": wq_c,
                "wv": wv_c,
                "cpak": np.ascontiguousarray(cpak.astype(f32)),
                "ident": ident,
            }
        )
    return in_maps


def assemble_out(results):
    out = np.empty((B, S, D), dtype=np.float32)
    for c in range(8):
        b, j = c // 2, c % 2
        blocks = J_BLOCKS[j]
        oc = results[c]["out_c"]  # (8, 128, 1024)
        for t, g in enumerate(blocks):
            out[b, P * g : P * (g + 1), :] = oc[t]
    return out


def kernel(x, Wq, bq, Wk, bk, Wv, bv, Wo, bo):
    from concourse.bass_utils import run_bass_kernel_spmd

    nc = build_nc()
    in_maps = make_in_maps(x, Wq, bq, Wk, bk, Wv, bv, Wo, bo)
    res = run_bass_kernel_spmd(nc, in_maps, core_ids=list(range(8)))
    return assemble_out(res.results)
